# revision 1
# baseline (speedup 1.0000x reference)
"""Trainium2 Bass kernel for DifferentiableRGBtoVel (soft-nearest-neighbor
colormap inversion).

velocity(p) = sum_k v_k e^{-100 d_k(p)} / sum_k e^{-100 d_k(p)},
d_k(p) = |p - c_k|^2.

Softmax stabilizer: the linear surrogate B_p = 100*sum_c(p_c) - 37.5 of
100|p|^2 (minimax linear fit of x^2 on [0,1]) keeps every exponent inside
fp32 range; the shift cancels exactly in the num/den ratio.

All matmuls are genuine fp32 (float32r truncates operands to 11 mantissa
bits, and mixing f32r with fp32 matmuls corrupts the fp32 ones on this
silicon -- measured, deterministic, schedule-dependent).

Per-core pipeline in [k, pix] layout (partition = colormap index), tiles of
512 pixels ([128, 1024] PSUM = [A|B]):
  scores: one K=4 fp32 matmul per 128-color half; halves run concurrently in
          PE row groups 0/32 (image rows duplicated to partitions 32-35).
          psum[k,px] = sum_c (c_kc-0.5) p_c + (37.5-100|c_k|^2)/200 * 1
  exp:    ONE ACT instruction per tile (FD=1024), func=Exp, scale=200.
  num/den: fp32 [128,2] matmuls (cols = [1, v_k]); each tile's chain rotates
          over the 4 PE column groups so up to 4 chains run concurrently.
          Output lands in a corner of the already-consumed score PSUM tile.
  divide: DVE copy -> SBUF, partition-compacting DMA into dense [128,512]
          accumulators (den rows 0-63, num rows 64-127), DVE reciprocal +
          multiply per 64-tile group, one output DMA per group.
"""

import numpy as np

import concourse.bass as bass
import concourse.mybir as mybir
import concourse.tile as tile_mod
from concourse.tile import TileContext
from concourse.vector_clock import ScopedClock, VectorClock
from concourse.bass_utils import run_bass_kernel_spmd

# ---------------------------------------------------------------- constants
N_CORES = 8
NB, C, H, W = 4, 3, 512, 512
K = 256
KH = 128
PIX_PER_CORE = NB * H * W // N_CORES   # 131072
TILE_PIX = 512                 # pixels per tile
GROUP = 64                     # tiles per division group
IMG_BATCH = 8                  # tiles per image DMA

_FP32 = mybir.dt.float32


# ------------------------------------------------- walrus sync-wait limits
# This walrus build rejects instructions carrying more than one sem wait
# ("Too many sync wait commands"); split extras onto same-engine NoOps.
def _split_drain_and_barrier(self, tick_clock, wait_clock):
    nc = self.nc
    vec = list(tick_clock.global_clock)
    for i, v in enumerate(vec):
        if v > 0:
            w = [0] * len(vec)
            w[i] = v
            inst = nc.sync.nop(nofuse=True, hint="split_drain_wait")
            wait_clock.add_sem_waits(inst.ins, ScopedClock({None: VectorClock(w)}))
    nc.sync.drain()
    nc.all_engine_barrier()
    assert self.sems is not None
    popped = nc._tile_sem_poison_stack.pop()
    assert popped is self._sem_poison
    nc.clear_and_free_semaphores(list(self.sems.allocated().values()))
    nc.all_engine_barrier()


tile_mod.TileContext._drain_and_barrier = _split_drain_and_barrier

MAX_WAITS = 1


def _split_excess_waits(nc, maxw=MAX_WAITS):
    for f in nc.m.functions:
        for bb in f.blocks:
            out = []
            for inst in bb.instructions:
                si = inst.sync_info
                if si is not None and len(si.on_wait) > maxw:
                    waits = list(si.on_wait)
                    excess, keep = waits[:-maxw], waits[-maxw:]
                    for i in range(0, len(excess), maxw):
                        nop = mybir.InstNoOp(
                            name=nc.get_next_instruction_name(),
                            sync_info=mybir.SyncInfo(
                                on_wait=excess[i:i + maxw], on_update=[]),
                            bass_nofuse=True,
                            engine=inst.engine,
                        )
                        out.append(nop)
                    inst.sync_info = mybir.SyncInfo(
                        on_wait=keep, on_update=list(si.on_update))
                out.append(inst)
            bb.instructions = out


# ------------------------------------------------------------- bass builder
def build_kernel(pix_per_core: int = PIX_PER_CORE):
    n_tiles = pix_per_core // TILE_PIX
    n_groups = (n_tiles + GROUP - 1) // GROUP

    nc = bass.Bass(trn_type="TRN2", name="rgb2vel")
    imgD = nc.dram_tensor("img", [4, pix_per_core], _FP32, kind="ExternalInput")
    cmD = nc.dram_tensor("cmt", [4, K], _FP32, kind="ExternalInput")
    vmD = nc.dram_tensor("vmat", [KH, 4], _FP32, kind="ExternalInput")
    velD = nc.dram_tensor("vel", [pix_per_core // 512, 512], _FP32,
                          kind="ExternalOutput")

    ExpF = mybir.ActivationFunctionType.Exp

    with TileContext(nc) as tc:
        with (
            tc.tile_pool(name="const", bufs=1) as cpool,
            tc.tile_pool(name="img", bufs=3) as ipool,
            tc.tile_pool(name="exp", bufs=6) as epool,
            tc.tile_pool(name="stg", bufs=8) as stgpool,
            tc.tile_pool(name="acc", bufs=2) as accpool,
            tc.tile_pool(name="divp", bufs=2) as dpool,
            tc.tile_pool(name="score", bufs=4, space="PSUM") as spool,
        ):
            # persistent constants: cm rows 0-3 = half A, rows 32-35 = half B
            cm = cpool.tile([36, KH], _FP32, tag="cm")
            nc.sync.dma_start(cm[0:4, :], cmD[:, 0:KH])
            nc.sync.dma_start(cm[32:36, :], cmD[:, KH:K])
            vm = cpool.tile([KH, 4], _FP32, tag="vm")
            nc.sync.dma_start(vm[:], vmD[:])

            state = {"img": None, "pending": []}

            def emit_tail(dnv):
                # V chain + evacuation for the oldest pending tile
                t, j, ps, ex = state["pending"].pop(0)
                m = t % 4
                tp = (0, 32 * m) if m else None
                out = ps[32 * m:32 * m + 2, 0:TILE_PIX]
                nc.tensor.matmul(out, lhsT=vm[:, 0:2], rhs=ex[:, 0:TILE_PIX],
                                 start=True, stop=False, tile_position=tp)
                nc.tensor.matmul(out, lhsT=vm[:, 2:4],
                                 rhs=ex[:, TILE_PIX:2 * TILE_PIX],
                                 start=False, stop=True, tile_position=tp)
                stg = stgpool.tile([98, TILE_PIX], _FP32, tag="stg")
                nc.vector.tensor_copy(stg[32 * m:32 * m + 2, :], out)
                nc.sync.dma_start(dnv[:, j, :], stg[32 * m:32 * m + 2, :])

            def do_tile(t, dnv, j):
                if t % IMG_BATCH == 0:
                    imgt = ipool.tile([36, IMG_BATCH * TILE_PIX], _FP32,
                                      tag="img")
                    sl = slice(t * TILE_PIX, (t + IMG_BATCH) * TILE_PIX)
                    nc.sync.dma_start(imgt[0:4, :], imgD[:, sl])
                    nc.sync.dma_start(imgt[32:36, :], imgD[:, sl])
                    state["img"] = imgt
                img = state["img"]
                ioff = (t % IMG_BATCH) * TILE_PIX
                isl = slice(ioff, ioff + TILE_PIX)

                # scores [A|B], halves concurrent in row groups 0/32
                ps = spool.tile([128, 2 * TILE_PIX], _FP32, tag="score")
                nc.tensor.matmul(ps[:, 0:TILE_PIX], lhsT=cm[0:4, :],
                                 rhs=img[0:4, isl], start=True, stop=True)
                nc.tensor.matmul(ps[:, TILE_PIX:2 * TILE_PIX],
                                 lhsT=cm[32:36, :], rhs=img[32:36, isl],
                                 start=True, stop=True)

                ex = epool.tile([128, 2 * TILE_PIX], _FP32, tag="exp")
                nc.scalar.activation(ex[:], ps[:], ExpF, bias=0.0, scale=200.0)
                state["pending"].append((t, j, ps, ex))
                if len(state["pending"]) >= 3:
                    # emit two V chains back-to-back: consecutive tiles use
                    # different PE column groups, so adjacent chains overlap
                    emit_tail(dnv)
                    emit_tail(dnv)

            def do_group(g, gtiles):
                dn = accpool.tile([128, 512], _FP32, tag="dn")
                dnv = dn.rearrange("(a p) w -> a p w", a=2)
                for j in range(gtiles):
                    do_tile(g * GROUP + j, dnv, j)
                while state["pending"]:
                    emit_tail(dnv)
                rows = gtiles
                nsh = dpool.tile([64, 512], _FP32, tag="nsh")
                nc.sync.dma_start(nsh[0:rows, :], dn[64:64 + rows, :])
                rcp = dpool.tile([64, 512], _FP32, tag="rcp")
                nc.vector.reciprocal(rcp[0:rows, :], dn[0:rows, :])
                vel = dpool.tile([64, 512], _FP32, tag="vel")
                nc.vector.tensor_tensor(
                    vel[0:rows, :], nsh[0:rows, :], rcp[0:rows, :],
                    mybir.AluOpType.mult)
                nc.sync.dma_start(velD[g * GROUP:g * GROUP + rows, :],
                                  vel[0:rows, :])

            for g in range(n_groups):
                do_group(g, min(GROUP, n_tiles - g * GROUP))

    _split_excess_waits(nc)
    return nc


# ----------------------------------------------------------- host wrapper
_CACHE = {}


def _get_nc(pix_per_core):
    if pix_per_core not in _CACHE:
        _CACHE[pix_per_core] = build_kernel(pix_per_core)
    return _CACHE[pix_per_core]


def _prep_consts(cmap, v_i):
    cmap = np.asarray(cmap, np.float32)
    v_i = np.asarray(v_i, np.float32)
    c2 = np.sum(cmap * cmap, axis=1, dtype=np.float32)
    cmt = np.empty((4, K), np.float32)
    cmt[0:3, :] = (cmap.T - np.float32(0.5))
    cmt[3, :] = (np.float32(37.5) - np.float32(100.0) * c2) / np.float32(200.0)
    vmat = np.empty((KH, 4), np.float32)
    vmat[:, 0] = 1.0
    vmat[:, 1] = v_i[0:KH]
    vmat[:, 2] = 1.0
    vmat[:, 3] = v_i[KH:K]
    return cmt, vmat


def _prep_image_slab(slab):
    """slab: [3, n] float32 -> [4, n] rows [r, g, b, ones]."""
    n = slab.shape[1]
    img = np.empty((4, n), np.float32)
    img[0:3] = slab
    img[3] = 1.0
    return img


def _kernel_impl(image, cmap, v_i, _trace=False):
    image = np.ascontiguousarray(np.asarray(image, np.float32))
    cmt, vmat = _prep_consts(cmap, v_i)

    rows_per_core = NB * H // N_CORES          # 256 rows of H per core
    in_maps = []
    for i in range(N_CORES):
        n = (i * rows_per_core) // H
        h0 = (i * rows_per_core) % H
        slab = image[n, :, h0:h0 + rows_per_core, :].reshape(3, -1)
        in_maps.append({"img": _prep_image_slab(slab), "cmt": cmt,
                        "vmat": vmat})

    nc = _get_nc(PIX_PER_CORE)
    res = run_bass_kernel_spmd(nc, in_maps, core_ids=list(range(N_CORES)),
                               trace=_trace)
    out = np.empty((NB, H, W), np.float32)
    for i in range(N_CORES):
        n = (i * rows_per_core) // H
        h0 = (i * rows_per_core) % H
        out[n, h0:h0 + rows_per_core, :] = \
            res.results[i]["vel"].reshape(rows_per_core, W)
    return out, res


def kernel(image, cmap, v_i):
    out, _ = _kernel_impl(image, cmap, v_i)
    return out



# revision 25
# speedup vs baseline: 1.0396x; 1.0396x over previous
"""Trainium2 Bass kernel for DifferentiableRGBtoVel (soft-nearest-neighbor
colormap inversion).

velocity(p) = sum_k v_k e^{-100 d_k(p)} / sum_k e^{-100 d_k(p)},
d_k(p) = |p - c_k|^2.

Softmax stabilizer: the linear surrogate B_p = 100*sum_c(p_c) - 37.5 of
100|p|^2 (minimax linear fit of x^2 on [0,1]) keeps every exponent inside
fp32 range; the shift cancels exactly in the num/den ratio.

All matmuls are float32r (1 cycle/row vs fp32's 4 when the output free dim
is >= 256).  f32r truncates operands to 11 mantissa bits, so:
  * score matmul operands are hi/lo (Dekker) split on the host: image rows
    [r_hi,g_hi,b_hi,1, r_lo,g_lo,b_lo,1] against pre-11-bit-rounded weights
    [w,w] and bias [b_hi,b_lo] in ONE contraction-8 matmul per K-half
    (contraction depth is free: cost = streamed columns only).  Scores are
    then exact to ~fp32 level for the colormap c'' = w + 0.5, which is
    within 2^-12/channel of the true colormap.
  * the num/den matmul truncates only the exp weights and v table
    (~5e-4 relative -> ~1e-3 output error; tolerance is 2e-2).

Per-core pipeline in [k, pix] layout (partition = colormap index), PAIRS of
512-pixel tiles share one [128, 2048] PSUM buffer (= 4 banks, 2 bufs):
  scores: one contraction-8 f32r matmul per 128-color half per tile;
          halves run concurrently in PE row groups 0/32.
  exp:    ONE ACT instruction per pair (FD=2048), func=Exp, scale=200 --
          amortizes the ~350-cycle ACT pipeline fill.
  num/den: f32r [128,2] matmuls (cols = [1, v_k]); both tiles of a pair
          land contiguously in one col group ([2,1024] at partitions 32m),
          rotating m over pairs so adjacent pairs' chains overlap.
  divide: one DVE copy [2,1024] -> SBUF staging, one partition-compacting
          DMA into dense [128,512] accumulators (den rows 0-63, num rows
          64-127), DVE reciprocal + multiply per 64-tile group, one output
          DMA per group.
"""

import numpy as np

import concourse.bass as bass
import concourse.mybir as mybir
import concourse.tile as tile_mod
from concourse.tile import TileContext
from concourse.vector_clock import ScopedClock, VectorClock
from concourse.bass_utils import run_bass_kernel_spmd

# ---------------------------------------------------------------- constants
N_CORES = 8
NB, C, H, W = 4, 3, 512, 512
K = 256
KH = 128
PIX_PER_CORE = NB * H * W // N_CORES   # 131072
TILE_PIX = 512                 # pixels per tile
PAIR_PIX = 2 * TILE_PIX        # pixels per PSUM/ACT pair
GROUP = 64                     # tiles per division group
IMG_BATCH = 8                  # tiles per image DMA

_FP32 = mybir.dt.float32
_F32R = mybir.dt.float32r


# ------------------------------------------------- walrus sync-wait limits
# This walrus build rejects instructions carrying more than one sem wait
# ("Too many sync wait commands"); split extras onto same-engine NoOps.
def _split_drain_and_barrier(self, tick_clock, wait_clock):
    nc = self.nc
    vec = list(tick_clock.global_clock)
    for i, v in enumerate(vec):
        if v > 0:
            w = [0] * len(vec)
            w[i] = v
            inst = nc.sync.nop(nofuse=True, hint="split_drain_wait")
            wait_clock.add_sem_waits(inst.ins, ScopedClock({None: VectorClock(w)}))
    nc.sync.drain()
    nc.all_engine_barrier()
    assert self.sems is not None
    popped = nc._tile_sem_poison_stack.pop()
    assert popped is self._sem_poison
    nc.clear_and_free_semaphores(list(self.sems.allocated().values()))
    nc.all_engine_barrier()


tile_mod.TileContext._drain_and_barrier = _split_drain_and_barrier

MAX_WAITS = 1


def _split_excess_waits(nc, maxw=MAX_WAITS):
    for f in nc.m.functions:
        for bb in f.blocks:
            out = []
            for inst in bb.instructions:
                si = inst.sync_info
                if si is not None and len(si.on_wait) > maxw:
                    waits = list(si.on_wait)
                    excess, keep = waits[:-maxw], waits[-maxw:]
                    for i in range(0, len(excess), maxw):
                        nop = mybir.InstNoOp(
                            name=nc.get_next_instruction_name(),
                            sync_info=mybir.SyncInfo(
                                on_wait=excess[i:i + maxw], on_update=[]),
                            bass_nofuse=True,
                            engine=inst.engine,
                        )
                        out.append(nop)
                    inst.sync_info = mybir.SyncInfo(
                        on_wait=keep, on_update=list(si.on_update))
                out.append(inst)
            bb.instructions = out


# ------------------------------------------------------------- bass builder
def build_kernel(pix_per_core: int = PIX_PER_CORE, split_waits: bool = True):
    n_tiles = pix_per_core // TILE_PIX
    n_pairs = n_tiles // 2
    n_groups = (n_tiles + GROUP - 1) // GROUP

    nc = bass.Bass(trn_type="TRN2", name="rgb2vel")
    imgD = nc.dram_tensor("img", [16, pix_per_core], _F32R, kind="ExternalInput")
    cmD = nc.dram_tensor("cmt", [8, K], _F32R, kind="ExternalInput")
    vmD = nc.dram_tensor("vmat", [KH, 4], _F32R, kind="ExternalInput")
    velD = nc.dram_tensor("vel", [pix_per_core // 512, 512], _FP32,
                          kind="ExternalOutput")

    ExpF = mybir.ActivationFunctionType.Exp

    with TileContext(nc) as tc:
        with (
            tc.tile_pool(name="const", bufs=1) as cpool,
            tc.tile_pool(name="img", bufs=3) as ipool,
            tc.tile_pool(name="exp", bufs=4) as epool,
            tc.tile_pool(name="stg", bufs=2) as stgpool,
            tc.tile_pool(name="acc", bufs=4) as accpool,
            tc.tile_pool(name="divp", bufs=4) as dpool,
            tc.tile_pool(name="score", bufs=2, space="PSUM") as spool,
        ):
            # persistent constants: cm rows 0-7 = half A, rows 32-39 = half B
            cm = cpool.tile([40, KH], _F32R, tag="cm")
            nc.sync.dma_start(cm[0:8, :], cmD[:, 0:KH])
            nc.sync.dma_start(cm[32:40, :], cmD[:, KH:K])
            vm = cpool.tile([KH, 4], _F32R, tag="vm")
            nc.sync.dma_start(vm[:], vmD[:])
            cmA = cm[0:8, :]
            cmB = cm[32:40, :]
            vmr = vm[:]

            state = {"img": None, "pending": [], "stg": None, "stg_n": 0,
                     "stg_j0": 0}
            STG_PAIRS = 8   # pairs per scatter DMA (keeps the division's
                            # DMA-queue wait list under the 7-sem HW cap)

            def emit_tail():
                # V chains + evacuation for the oldest pending pair.
                # f32r matmul outputs must start at PSUM partition 0
                # (s3d3_mm_valid_dst_partition), so no col-group rotation.
                p, j0, ps, ex, dn = state["pending"].pop(0)
                exr = ex[:]
                out = ps[0:2, 0:PAIR_PIX]
                for q in range(2):
                    o = out[:, q * TILE_PIX:(q + 1) * TILE_PIX]
                    e0 = q * PAIR_PIX
                    nc.tensor.matmul(o, lhsT=vmr[:, 0:2],
                                     rhs=exr[:, e0:e0 + TILE_PIX],
                                     start=True, stop=False)
                    nc.tensor.matmul(o, lhsT=vmr[:, 2:4],
                                     rhs=exr[:, e0 + TILE_PIX:e0 + PAIR_PIX],
                                     start=False, stop=True)
                if state["stg"] is None:
                    stg_t = stgpool.tile(
                        [2, STG_PAIRS * PAIR_PIX], _FP32, tag="stg")
                    state["stg"] = stg_t
                    state["stg_n"] = 0
                    state["stg_j0"] = j0
                stg = state["stg"]
                off = state["stg_n"] * PAIR_PIX
                nc.vector.tensor_copy(stg[:, off:off + PAIR_PIX], out)
                state["stg_n"] += 1
                if state["stg_n"] == STG_PAIRS:
                    # stg row 0 = dens of 16 tiles, row 1 = nums.  Plain
                    # partition-slice destinations (a rearranged multi-
                    # partition-dim view mis-lowers its offset and clobbers
                    # low SBUF addresses).  Two DMAs per batch also keeps
                    # each division reader's queue-wait list under the
                    # 7-semaphore instruction cap.
                    j0b = state["stg_j0"]
                    nt = 2 * STG_PAIRS
                    nc.sync.dma_start(dn[j0b:j0b + nt, :], stg[0:1, :])
                    nc.sync.dma_start(dn[64 + j0b:64 + j0b + nt, :],
                                      stg[1:2, :])
                    state["stg"] = None

            def do_pair(p, dn, j0):
                t0 = 2 * p
                if t0 % IMG_BATCH == 0:
                    imgt = ipool.tile([40, IMG_BATCH * TILE_PIX], _F32R,
                                      tag="img")
                    sl = slice(t0 * TILE_PIX, (t0 + IMG_BATCH) * TILE_PIX)
                    nc.sync.dma_start(imgt[0:8, :], imgD[0:8, sl])
                    nc.sync.dma_start(imgt[32:40, :], imgD[8:16, sl])
                    state["img"] = imgt
                img = state["img"]

                # Emit the oldest pending pair's V chain BEFORE allocating a
                # new PSUM tile: spool has 2 slots, so at most 2 pairs may be
                # alive or the scheduler is forced into a serial schedule.
                while len(state["pending"]) >= 2:
                    emit_tail()

                ps = spool.tile([128, 2 * PAIR_PIX], _FP32, tag="score")
                for q in range(2):
                    ioff = ((t0 + q) % IMG_BATCH) * TILE_PIX
                    ra = img[0:8, ioff:ioff + TILE_PIX]
                    rb = img[32:40, ioff:ioff + TILE_PIX]
                    o0 = q * PAIR_PIX
                    nc.tensor.matmul(ps[:, o0:o0 + TILE_PIX], lhsT=cmA,
                                     rhs=ra, start=True, stop=True)
                    nc.tensor.matmul(ps[:, o0 + TILE_PIX:o0 + PAIR_PIX],
                                     lhsT=cmB, rhs=rb, start=True, stop=True)

                ex = epool.tile([128, 2 * PAIR_PIX], _F32R, tag="exp")
                nc.scalar.activation(ex[:], ps[:], ExpF, bias=0.0, scale=200.0)
                state["pending"].append((p, j0, ps, ex, dn))

            def do_group(g, gtiles):
                dn = accpool.tile([128, 512], _FP32, tag="dn")
                for jp in range(gtiles // 2):
                    do_pair((g * GROUP) // 2 + jp, dn, 2 * jp)
                while state["pending"]:
                    emit_tail()
                rows = gtiles
                nsh = dpool.tile([64, 512], _FP32, tag="nsh")
                nc.sync.dma_start(nsh[0:rows, :], dn[64:64 + rows, :])
                rcp = dpool.tile([64, 512], _FP32, tag="rcp")
                nc.vector.reciprocal(rcp[0:rows, :], dn[0:rows, :])
                vel = dpool.tile([64, 512], _FP32, tag="vel")
                nc.vector.tensor_tensor(
                    vel[0:rows, :], nsh[0:rows, :], rcp[0:rows, :],
                    mybir.AluOpType.mult)
                nc.sync.dma_start(velD[g * GROUP:g * GROUP + rows, :],
                                  vel[0:rows, :])

            for g in range(n_groups):
                do_group(g, min(GROUP, n_tiles - g * GROUP))

    if split_waits:
        _split_excess_waits(nc)
    return nc


# ----------------------------------------------------------- host wrapper
_CACHE = {}


def _get_nc(pix_per_core):
    if pix_per_core not in _CACHE:
        _CACHE[pix_per_core] = build_kernel(pix_per_core)
    return _CACHE[pix_per_core]


def _r11(x):
    """Round fp32 array to 11 significant mantissa bits (f32r precision)."""
    u = np.ascontiguousarray(np.asarray(x, np.float32)).view(np.uint32)
    u = (u + np.uint32(0x1000)) & np.uint32(0xFFFFE000)
    return u.view(np.float32)


def _prep_consts(cmap, v_i):
    cmap = np.asarray(cmap, np.float64)
    v_i = np.asarray(v_i, np.float32)
    w = _r11(np.float32(cmap - 0.5))             # [K,3] 11-bit weights
    cpp = w.astype(np.float64) + 0.5             # effective colormap c''
    c2 = np.sum(cpp * cpp, axis=1)               # fp64
    b = (37.5 - 100.0 * c2) / 200.0              # fp64 [K]
    b_hi = _r11(np.float32(b))
    b_lo = np.float32(b - b_hi.astype(np.float64))
    cmt = np.empty((8, K), np.float32)
    cmt[0:3, :] = w.T
    cmt[3, :] = b_hi
    cmt[4:7, :] = w.T
    cmt[7, :] = b_lo
    vmat = np.empty((KH, 4), np.float32)
    vmat[:, 0] = 1.0
    vmat[:, 1] = v_i[0:KH]
    vmat[:, 2] = 1.0
    vmat[:, 3] = v_i[KH:K]
    return cmt, vmat


def _prep_image_slab(slab):
    """slab: [3, n] float32 -> [16, n] rows [hi(3),1,lo(3),1] twice."""
    n = slab.shape[1]
    hi = _r11(slab)
    lo = np.float32(np.asarray(slab, np.float32) - hi)
    img = np.empty((16, n), np.float32)
    img[0:3] = hi
    img[3] = 1.0
    img[4:7] = lo
    img[7] = 1.0
    img[8:16] = img[0:8]
    return img


def _kernel_impl(image, cmap, v_i, _trace=False):
    image = np.ascontiguousarray(np.asarray(image, np.float32))
    cmt, vmat = _prep_consts(cmap, v_i)

    rows_per_core = NB * H // N_CORES          # 256 rows of H per core
    in_maps = []
    for i in range(N_CORES):
        n = (i * rows_per_core) // H
        h0 = (i * rows_per_core) % H
        slab = image[n, :, h0:h0 + rows_per_core, :].reshape(3, -1)
        in_maps.append({"img": _prep_image_slab(slab), "cmt": cmt,
                        "vmat": vmat})

    nc = _get_nc(PIX_PER_CORE)
    res = run_bass_kernel_spmd(nc, in_maps, core_ids=list(range(N_CORES)),
                               trace=_trace)
    out = np.empty((NB, H, W), np.float32)
    for i in range(N_CORES):
        n = (i * rows_per_core) // H
        h0 = (i * rows_per_core) % H
        out[n, h0:h0 + rows_per_core, :] = \
            res.results[i]["vel"].reshape(rows_per_core, W)
    return out, res


def kernel(image, cmap, v_i):
    out, _ = _kernel_impl(image, cmap, v_i)
    return out


# revision 27
# speedup vs baseline: 1.1837x; 1.1386x over previous
"""Trainium2 Bass kernel for DifferentiableRGBtoVel (soft-nearest-neighbor
colormap inversion).

velocity(p) = sum_k v_k e^{-100 d_k(p)} / sum_k e^{-100 d_k(p)},
d_k(p) = |p - c_k|^2.

Softmax stabilizer: the linear surrogate B_p = 100*sum_c(p_c) - 37.5 of
100|p|^2 (minimax linear fit of x^2 on [0,1]) keeps every exponent inside
fp32 range; the shift cancels exactly in the num/den ratio.

All matmuls are float32r (1 cycle/row vs fp32's 4 when the output free dim
is >= 256).  f32r truncates operands to 11 mantissa bits, so:
  * score matmul operands are hi/lo (Dekker) split on the host: image rows
    [r_hi,g_hi,b_hi,1, r_lo,g_lo,b_lo,1] against pre-11-bit-rounded weights
    [w,w] and bias [b_hi,b_lo] in ONE contraction-8 matmul per K-half
    (contraction depth is free: cost = streamed columns only).  Scores are
    then exact to ~fp32 level for the colormap c'' = w + 0.5, which is
    within 2^-12/channel of the true colormap.
  * the num/den matmul truncates only the exp weights and v table
    (~5e-4 relative -> ~1e-3 output error; tolerance is 2e-2).

Per-core pipeline in [k, pix] layout (partition = colormap index), PAIRS of
512-pixel tiles share one [128, 2048] PSUM buffer (= 4 banks, 2 bufs):
  scores: one contraction-8 f32r matmul per 128-color half per tile;
          halves run concurrently in PE row groups 0/32.
  exp:    ONE ACT instruction per pair (FD=2048), func=Exp, scale=200 --
          amortizes the ~350-cycle ACT pipeline fill.
  num/den: f32r [128,2] matmuls (cols = [1, v_k]); both tiles of a pair
          land contiguously in one col group ([2,1024] at partitions 32m),
          rotating m over pairs so adjacent pairs' chains overlap.
  divide: one DVE copy [2,1024] -> SBUF staging, one partition-compacting
          DMA into dense [128,512] accumulators (den rows 0-63, num rows
          64-127), DVE reciprocal + multiply per 64-tile group, one output
          DMA per group.
"""

import numpy as np

import concourse.bass as bass
import concourse.mybir as mybir
import concourse.tile as tile_mod
from concourse.tile import TileContext
from concourse.vector_clock import ScopedClock, VectorClock
from concourse.bass_utils import run_bass_kernel_spmd

# ---------------------------------------------------------------- constants
N_CORES = 8
NB, C, H, W = 4, 3, 512, 512
K = 256
KH = 128
PIX_PER_CORE = NB * H * W // N_CORES   # 131072
TILE_PIX = 512                 # pixels per tile
PAIR_PIX = 2 * TILE_PIX        # pixels per PSUM/ACT pair
GROUP = 64                     # tiles per division group
IMG_BATCH = 8                  # tiles per image DMA

_FP32 = mybir.dt.float32
_F32R = mybir.dt.float32r


# ------------------------------------------------- walrus sync-wait limits
# This walrus build rejects instructions carrying more than one sem wait
# ("Too many sync wait commands"); split extras onto same-engine NoOps.
def _split_drain_and_barrier(self, tick_clock, wait_clock):
    nc = self.nc
    vec = list(tick_clock.global_clock)
    for i, v in enumerate(vec):
        if v > 0:
            w = [0] * len(vec)
            w[i] = v
            inst = nc.sync.nop(nofuse=True, hint="split_drain_wait")
            wait_clock.add_sem_waits(inst.ins, ScopedClock({None: VectorClock(w)}))
    nc.sync.drain()
    nc.all_engine_barrier()
    assert self.sems is not None
    popped = nc._tile_sem_poison_stack.pop()
    assert popped is self._sem_poison
    nc.clear_and_free_semaphores(list(self.sems.allocated().values()))
    nc.all_engine_barrier()


tile_mod.TileContext._drain_and_barrier = _split_drain_and_barrier

MAX_WAITS = 1


def _split_excess_waits(nc, maxw=MAX_WAITS):
    for f in nc.m.functions:
        for bb in f.blocks:
            out = []
            for inst in bb.instructions:
                si = inst.sync_info
                if si is not None and len(si.on_wait) > maxw:
                    waits = list(si.on_wait)
                    excess, keep = waits[:-maxw], waits[-maxw:]
                    for i in range(0, len(excess), maxw):
                        nop = mybir.InstNoOp(
                            name=nc.get_next_instruction_name(),
                            sync_info=mybir.SyncInfo(
                                on_wait=excess[i:i + maxw], on_update=[]),
                            bass_nofuse=True,
                            engine=inst.engine,
                        )
                        out.append(nop)
                    inst.sync_info = mybir.SyncInfo(
                        on_wait=keep, on_update=list(si.on_update))
                out.append(inst)
            bb.instructions = out


# ------------------------------------------------------------- bass builder
def build_kernel(pix_per_core: int = PIX_PER_CORE, split_waits: bool = True):
    n_tiles = pix_per_core // TILE_PIX
    n_pairs = n_tiles // 2
    n_groups = (n_tiles + GROUP - 1) // GROUP

    nc = bass.Bass(trn_type="TRN2", name="rgb2vel")
    imgD = nc.dram_tensor("img", [16, pix_per_core], _F32R, kind="ExternalInput")
    cmD = nc.dram_tensor("cmt", [8, K], _F32R, kind="ExternalInput")
    vmD = nc.dram_tensor("vmat", [KH, 4], _F32R, kind="ExternalInput")
    velD = nc.dram_tensor("vel", [pix_per_core // 512, 512], _FP32,
                          kind="ExternalOutput")

    ExpF = mybir.ActivationFunctionType.Exp

    with TileContext(nc) as tc:
        with (
            tc.tile_pool(name="const", bufs=1) as cpool,
            tc.tile_pool(name="img", bufs=3) as ipool,
            tc.tile_pool(name="exp", bufs=6) as epool,
            tc.tile_pool(name="stg", bufs=2) as stgpool,
            tc.tile_pool(name="acc", bufs=4) as accpool,
            tc.tile_pool(name="divp", bufs=4) as dpool,
            tc.tile_pool(name="score", bufs=4, space="PSUM") as spool,
        ):
            # persistent constants: cm rows 0-7 = half A, rows 32-39 = half B
            cm = cpool.tile([40, KH], _F32R, tag="cm")
            nc.sync.dma_start(cm[0:8, :], cmD[:, 0:KH])
            nc.sync.dma_start(cm[32:40, :], cmD[:, KH:K])
            vm = cpool.tile([KH, 4], _F32R, tag="vm")
            nc.sync.dma_start(vm[:], vmD[:])
            cmA = cm[0:8, :]
            cmB = cm[32:40, :]
            vmr = vm[:]

            state = {"img": None, "pending": [], "stg": None, "stg_n": 0,
                     "stg_j0": 0}
            STG_TILES = 16  # tiles per scatter DMA (keeps the division's
                            # DMA-queue wait list under the 7-sem HW cap)

            def emit_tail():
                # V chain + evacuation for the oldest pending tile.
                # f32r matmul outputs must start at PSUM partition 0
                # (s3d3_mm_valid_dst_partition), so no col-group rotation.
                t, j, ps, ex, dn = state["pending"].pop(0)
                out = ps[0:2, 0:TILE_PIX]
                nc.tensor.matmul(out, lhsT=vmr[:, 0:2],
                                 rhs=ex[:, 0:TILE_PIX],
                                 start=True, stop=False)
                nc.tensor.matmul(out, lhsT=vmr[:, 2:4],
                                 rhs=ex[:, TILE_PIX:2 * TILE_PIX],
                                 start=False, stop=True)
                if state["stg"] is None:
                    stg_t = stgpool.tile(
                        [2, STG_TILES * TILE_PIX], _FP32, tag="stg")
                    state["stg"] = stg_t
                    state["stg_n"] = 0
                    state["stg_j0"] = j
                stg = state["stg"]
                off = state["stg_n"] * TILE_PIX
                nc.vector.tensor_copy(stg[:, off:off + TILE_PIX], out)
                state["stg_n"] += 1
                if state["stg_n"] == STG_TILES:
                    # stg row 0 = dens of 16 tiles, row 1 = nums.  Plain
                    # partition-slice destinations (a rearranged multi-
                    # partition-dim view mis-lowers its offset and clobbers
                    # low SBUF addresses).  Two DMAs per batch also keeps
                    # each division reader's queue-wait list under the
                    # 7-semaphore instruction cap.
                    j0b = state["stg_j0"]
                    nc.sync.dma_start(dn[j0b:j0b + STG_TILES, :], stg[0:1, :])
                    nc.sync.dma_start(dn[64 + j0b:64 + j0b + STG_TILES, :],
                                      stg[1:2, :])
                    state["stg"] = None

            def do_tile(t, dn, j):
                if t % IMG_BATCH == 0:
                    imgt = ipool.tile([40, IMG_BATCH * TILE_PIX], _F32R,
                                      tag="img")
                    sl = slice(t * TILE_PIX, (t + IMG_BATCH) * TILE_PIX)
                    nc.sync.dma_start(imgt[0:8, :], imgD[0:8, sl])
                    nc.sync.dma_start(imgt[32:40, :], imgD[8:16, sl])
                    state["img"] = imgt
                img = state["img"]

                # Emit the oldest pending tile's V chain BEFORE allocating a
                # new PSUM tile: spool has 4 slots, so at most 4 tiles may be
                # alive or the scheduler is forced into a serial schedule.
                while len(state["pending"]) >= 3:
                    emit_tail()

                ioff = (t % IMG_BATCH) * TILE_PIX
                ra = img[0:8, ioff:ioff + TILE_PIX]
                rb = img[32:40, ioff:ioff + TILE_PIX]
                ps = spool.tile([128, 2 * TILE_PIX], _FP32, tag="score")
                nc.tensor.matmul(ps[:, 0:TILE_PIX], lhsT=cmA,
                                 rhs=ra, start=True, stop=True)
                nc.tensor.matmul(ps[:, TILE_PIX:2 * TILE_PIX],
                                 lhsT=cmB, rhs=rb, start=True, stop=True)

                ex = epool.tile([128, 2 * TILE_PIX], _F32R, tag="exp")
                nc.scalar.activation(ex[:], ps[:], ExpF, bias=0.0, scale=200.0)
                state["pending"].append((t, j, ps, ex, dn))

            def do_group(g, gtiles):
                dn = accpool.tile([128, 512], _FP32, tag="dn")
                for j in range(gtiles):
                    do_tile(g * GROUP + j, dn, j)
                while state["pending"]:
                    emit_tail()
                rows = gtiles
                nsh = dpool.tile([64, 512], _FP32, tag="nsh")
                nc.sync.dma_start(nsh[0:rows, :], dn[64:64 + rows, :])
                rcp = dpool.tile([64, 512], _FP32, tag="rcp")
                nc.vector.reciprocal(rcp[0:rows, :], dn[0:rows, :])
                vel = dpool.tile([64, 512], _FP32, tag="vel")
                nc.vector.tensor_tensor(
                    vel[0:rows, :], nsh[0:rows, :], rcp[0:rows, :],
                    mybir.AluOpType.mult)
                nc.sync.dma_start(velD[g * GROUP:g * GROUP + rows, :],
                                  vel[0:rows, :])

            for g in range(n_groups):
                do_group(g, min(GROUP, n_tiles - g * GROUP))

    if split_waits:
        _split_excess_waits(nc)
    return nc


# ----------------------------------------------------------- host wrapper
_CACHE = {}


def _get_nc(pix_per_core):
    if pix_per_core not in _CACHE:
        _CACHE[pix_per_core] = build_kernel(pix_per_core)
    return _CACHE[pix_per_core]


def _r11(x):
    """Round fp32 array to 11 significant mantissa bits (f32r precision)."""
    u = np.ascontiguousarray(np.asarray(x, np.float32)).view(np.uint32)
    u = (u + np.uint32(0x1000)) & np.uint32(0xFFFFE000)
    return u.view(np.float32)


def _prep_consts(cmap, v_i):
    cmap = np.asarray(cmap, np.float64)
    v_i = np.asarray(v_i, np.float32)
    w = _r11(np.float32(cmap - 0.5))             # [K,3] 11-bit weights
    cpp = w.astype(np.float64) + 0.5             # effective colormap c''
    c2 = np.sum(cpp * cpp, axis=1)               # fp64
    b = (37.5 - 100.0 * c2) / 200.0              # fp64 [K]
    b_hi = _r11(np.float32(b))
    b_lo = np.float32(b - b_hi.astype(np.float64))
    cmt = np.empty((8, K), np.float32)
    cmt[0:3, :] = w.T
    cmt[3, :] = b_hi
    cmt[4:7, :] = w.T
    cmt[7, :] = b_lo
    vmat = np.empty((KH, 4), np.float32)
    vmat[:, 0] = 1.0
    vmat[:, 1] = v_i[0:KH]
    vmat[:, 2] = 1.0
    vmat[:, 3] = v_i[KH:K]
    return cmt, vmat


def _prep_image_slab(slab):
    """slab: [3, n] float32 -> [16, n] rows [hi(3),1,lo(3),1] twice."""
    n = slab.shape[1]
    hi = _r11(slab)
    lo = np.float32(np.asarray(slab, np.float32) - hi)
    img = np.empty((16, n), np.float32)
    img[0:3] = hi
    img[3] = 1.0
    img[4:7] = lo
    img[7] = 1.0
    img[8:16] = img[0:8]
    return img


def _kernel_impl(image, cmap, v_i, _trace=False):
    image = np.ascontiguousarray(np.asarray(image, np.float32))
    cmt, vmat = _prep_consts(cmap, v_i)

    rows_per_core = NB * H // N_CORES          # 256 rows of H per core
    in_maps = []
    for i in range(N_CORES):
        n = (i * rows_per_core) // H
        h0 = (i * rows_per_core) % H
        slab = image[n, :, h0:h0 + rows_per_core, :].reshape(3, -1)
        in_maps.append({"img": _prep_image_slab(slab), "cmt": cmt,
                        "vmat": vmat})

    nc = _get_nc(PIX_PER_CORE)
    res = run_bass_kernel_spmd(nc, in_maps, core_ids=list(range(N_CORES)),
                               trace=_trace)
    out = np.empty((NB, H, W), np.float32)
    for i in range(N_CORES):
        n = (i * rows_per_core) // H
        h0 = (i * rows_per_core) % H
        out[n, h0:h0 + rows_per_core, :] = \
            res.results[i]["vel"].reshape(rows_per_core, W)
    return out, res


def kernel(image, cmap, v_i):
    out, _ = _kernel_impl(image, cmap, v_i)
    return out


# revision 28
# speedup vs baseline: 1.5732x; 1.3291x over previous
"""Trainium2 Bass kernel for DifferentiableRGBtoVel (soft-nearest-neighbor
colormap inversion).

velocity(p) = sum_k v_k e^{-100 d_k(p)} / sum_k e^{-100 d_k(p)},
d_k(p) = |p - c_k|^2.

Softmax stabilizer: the linear surrogate B_p = 100*sum_c(p_c) - 37.5 of
100|p|^2 (minimax linear fit of x^2 on [0,1]) keeps every exponent inside
fp32 range; the shift cancels exactly in the num/den ratio.

All matmuls are BF16 (the PE's native 1-cycle/row path with fast weight
load; fp32 runs at 4 cycles/row and float32r measures ~2 cycles/row on this
silicon AND keeps the HAM clock-gate cold).  Precision is recovered with
multi-term Dekker splits in the CONTRACTION dimension, which is free (cost =
streamed columns only): w = w1+w2+w3 and p = p1+p2+p3 (bf16 each), bias =
b1+b2+b3, keeping products (w1,p1..p3),(w2,p1..p2),(w3,p1),(bias,1) -> 21
contraction rows per K-half.  bf16 x bf16 products are exact in the fp32
PSUM accumulate, so scores are accurate to ~1e-5; only the exp table and
v_i table carry bf16 rounding (~4e-3 output error vs the 2e-2 gate).

Per-core pipeline in [k, pix] layout (partition = colormap index), tiles of
512 pixels ([128, 1024] PSUM = [A|B], 4-deep for scheduling slack):
  scores: one contraction-21 bf16 matmul per 128-color half; halves run
          concurrently in PE row groups 0/32 (A rows 0-20, B rows 32-52).
  exp:    ONE ACT instruction per tile (FD=1024), func=Exp, scale=200,
          bf16 output.
  num/den: bf16 [128,2] matmuls (cols = [1, v_k]); output [2,512] at PSUM
          partitions 0-1 in the tile's own consumed score region.
  divide: DVE copy [2,512] -> a [2, 16*512] staging tile; per 16 tiles TWO
          partition-slice DMAs redistribute dens -> dn rows j..j+15 and
          nums -> dn rows 64+j.. (plain slices only: a rearranged multi-
          partition-dim view mis-lowers its offset and clobbers low SBUF;
          2 DMAs/batch also keeps the division's queue-wait list under the
          7-semaphore instruction cap).  DVE reciprocal + multiply per
          64-tile group, one output DMA per group.
"""

import numpy as np
import ml_dtypes

import concourse.bass as bass
import concourse.mybir as mybir
import concourse.tile as tile_mod
from concourse.tile import TileContext
from concourse.vector_clock import ScopedClock, VectorClock
from concourse.bass_utils import run_bass_kernel_spmd

# ---------------------------------------------------------------- constants
N_CORES = 8
NB, C, H, W = 4, 3, 512, 512
K = 256
KH = 128
PIX_PER_CORE = NB * H * W // N_CORES   # 131072
TILE_PIX = 512                 # pixels per tile
GROUP = 64                     # tiles per division group
IMG_BATCH = 8                  # tiles per image DMA
NROW = 21                      # contraction rows per K-half

_FP32 = mybir.dt.float32
_BF16 = mybir.dt.bfloat16
_BF = ml_dtypes.bfloat16


# ------------------------------------------------- walrus sync-wait limits
# This walrus build rejects instructions carrying more than one sem wait
# ("Too many sync wait commands"); split extras onto same-engine NoOps.
def _split_drain_and_barrier(self, tick_clock, wait_clock):
    nc = self.nc
    vec = list(tick_clock.global_clock)
    for i, v in enumerate(vec):
        if v > 0:
            w = [0] * len(vec)
            w[i] = v
            inst = nc.sync.nop(nofuse=True, hint="split_drain_wait")
            wait_clock.add_sem_waits(inst.ins, ScopedClock({None: VectorClock(w)}))
    nc.sync.drain()
    nc.all_engine_barrier()
    assert self.sems is not None
    popped = nc._tile_sem_poison_stack.pop()
    assert popped is self._sem_poison
    nc.clear_and_free_semaphores(list(self.sems.allocated().values()))
    nc.all_engine_barrier()


tile_mod.TileContext._drain_and_barrier = _split_drain_and_barrier

MAX_WAITS = 1


def _split_excess_waits(nc, maxw=MAX_WAITS):
    for f in nc.m.functions:
        for bb in f.blocks:
            out = []
            for inst in bb.instructions:
                si = inst.sync_info
                if si is not None and len(si.on_wait) > maxw:
                    waits = list(si.on_wait)
                    excess, keep = waits[:-maxw], waits[-maxw:]
                    for i in range(0, len(excess), maxw):
                        nop = mybir.InstNoOp(
                            name=nc.get_next_instruction_name(),
                            sync_info=mybir.SyncInfo(
                                on_wait=excess[i:i + maxw], on_update=[]),
                            bass_nofuse=True,
                            engine=inst.engine,
                        )
                        out.append(nop)
                    inst.sync_info = mybir.SyncInfo(
                        on_wait=keep, on_update=list(si.on_update))
                out.append(inst)
            bb.instructions = out


# ------------------------------------------------------------- bass builder
def build_kernel(pix_per_core: int = PIX_PER_CORE, split_waits: bool = True):
    n_tiles = pix_per_core // TILE_PIX
    n_groups = (n_tiles + GROUP - 1) // GROUP

    nc = bass.Bass(trn_type="TRN2", name="rgb2vel")
    imgD = nc.dram_tensor("img", [2 * NROW, pix_per_core], _BF16,
                          kind="ExternalInput")
    cmD = nc.dram_tensor("cmt", [NROW, K], _BF16, kind="ExternalInput")
    vmD = nc.dram_tensor("vmat", [KH, 4], _BF16, kind="ExternalInput")
    velD = nc.dram_tensor("vel", [pix_per_core // 512, 512], _FP32,
                          kind="ExternalOutput")

    ExpF = mybir.ActivationFunctionType.Exp

    with TileContext(nc) as tc:
        with (
            tc.tile_pool(name="const", bufs=1) as cpool,
            tc.tile_pool(name="img", bufs=3) as ipool,
            tc.tile_pool(name="exp", bufs=6) as epool,
            tc.tile_pool(name="stg", bufs=2) as stgpool,
            tc.tile_pool(name="acc", bufs=4) as accpool,
            tc.tile_pool(name="divp", bufs=4) as dpool,
            tc.tile_pool(name="score", bufs=4, space="PSUM") as spool,
        ):
            # persistent constants: cm rows 0-20 = half A, 32-52 = half B
            cm = cpool.tile([32 + NROW, KH], _BF16, tag="cm")
            nc.sync.dma_start(cm[0:NROW, :], cmD[:, 0:KH])
            nc.sync.dma_start(cm[32:32 + NROW, :], cmD[:, KH:K])
            vm = cpool.tile([KH, 4], _BF16, tag="vm")
            nc.sync.dma_start(vm[:], vmD[:])
            cmA = cm[0:NROW, :]
            cmB = cm[32:32 + NROW, :]
            vmr = vm[:]

            state = {"img": None, "pending": [], "stg": None, "stg_n": 0,
                     "stg_j0": 0}
            STG_TILES = 16  # tiles per scatter DMA

            def emit_tail():
                # V chain + evacuation for the oldest pending tile.
                t, j, ps, ex, dn = state["pending"].pop(0)
                out = ps[0:2, 0:TILE_PIX]
                nc.tensor.matmul(out, lhsT=vmr[:, 0:2],
                                 rhs=ex[:, 0:TILE_PIX],
                                 start=True, stop=False)
                nc.tensor.matmul(out, lhsT=vmr[:, 2:4],
                                 rhs=ex[:, TILE_PIX:2 * TILE_PIX],
                                 start=False, stop=True)
                if state["stg"] is None:
                    stg_t = stgpool.tile(
                        [2, STG_TILES * TILE_PIX], _FP32, tag="stg")
                    state["stg"] = stg_t
                    state["stg_n"] = 0
                    state["stg_j0"] = j
                stg = state["stg"]
                off = state["stg_n"] * TILE_PIX
                nc.vector.tensor_copy(stg[:, off:off + TILE_PIX], out)
                state["stg_n"] += 1
                if state["stg_n"] == STG_TILES:
                    j0b = state["stg_j0"]
                    nc.sync.dma_start(dn[j0b:j0b + STG_TILES, :], stg[0:1, :])
                    nc.sync.dma_start(dn[64 + j0b:64 + j0b + STG_TILES, :],
                                      stg[1:2, :])
                    state["stg"] = None

            def do_tile(t, dn, j):
                if t % IMG_BATCH == 0:
                    imgt = ipool.tile([32 + NROW, IMG_BATCH * TILE_PIX],
                                      _BF16, tag="img")
                    sl = slice(t * TILE_PIX, (t + IMG_BATCH) * TILE_PIX)
                    nc.sync.dma_start(imgt[0:NROW, :], imgD[0:NROW, sl])
                    nc.sync.dma_start(imgt[32:32 + NROW, :],
                                      imgD[NROW:2 * NROW, sl])
                    state["img"] = imgt
                img = state["img"]

                # Emit the oldest pending tile's V chain BEFORE allocating a
                # new PSUM tile: spool has 4 slots, so at most 4 tiles may be
                # alive or the scheduler is forced into a serial schedule.
                while len(state["pending"]) >= 3:
                    emit_tail()

                ioff = (t % IMG_BATCH) * TILE_PIX
                ra = img[0:NROW, ioff:ioff + TILE_PIX]
                rb = img[32:32 + NROW, ioff:ioff + TILE_PIX]
                ps = spool.tile([128, 2 * TILE_PIX], _FP32, tag="score")
                nc.tensor.matmul(ps[:, 0:TILE_PIX], lhsT=cmA,
                                 rhs=ra, start=True, stop=True)
                nc.tensor.matmul(ps[:, TILE_PIX:2 * TILE_PIX],
                                 lhsT=cmB, rhs=rb, start=True, stop=True)

                ex = epool.tile([128, 2 * TILE_PIX], _BF16, tag="exp")
                nc.scalar.activation(ex[:], ps[:], ExpF, bias=0.0, scale=200.0)
                state["pending"].append((t, j, ps, ex, dn))

            def do_group(g, gtiles):
                dn = accpool.tile([128, 512], _FP32, tag="dn")
                for j in range(gtiles):
                    do_tile(g * GROUP + j, dn, j)
                while state["pending"]:
                    emit_tail()
                rows = gtiles
                nsh = dpool.tile([64, 512], _FP32, tag="nsh")
                nc.sync.dma_start(nsh[0:rows, :], dn[64:64 + rows, :])
                rcp = dpool.tile([64, 512], _FP32, tag="rcp")
                nc.vector.reciprocal(rcp[0:rows, :], dn[0:rows, :])
                vel = dpool.tile([64, 512], _FP32, tag="vel")
                nc.vector.tensor_tensor(
                    vel[0:rows, :], nsh[0:rows, :], rcp[0:rows, :],
                    mybir.AluOpType.mult)
                nc.sync.dma_start(velD[g * GROUP:g * GROUP + rows, :],
                                  vel[0:rows, :])

            for g in range(n_groups):
                do_group(g, min(GROUP, n_tiles - g * GROUP))

    if split_waits:
        _split_excess_waits(nc)
    return nc


# ----------------------------------------------------------- host wrapper
_CACHE = {}


def _get_nc(pix_per_core):
    if pix_per_core not in _CACHE:
        _CACHE[pix_per_core] = build_kernel(pix_per_core)
    return _CACHE[pix_per_core]


def _bf_splits(x, n):
    """n-term Dekker split of fp32 array into bf16 parts (sum == x to
    ~2^-8n relative)."""
    outs = []
    r = np.asarray(x, np.float32)
    for _ in range(n):
        b = r.astype(_BF)
        outs.append(b)
        r = np.float32(r - b.astype(np.float32))
    return outs


def _prep_consts(cmap, v_i):
    cmap = np.asarray(cmap, np.float64)
    v_i = np.asarray(v_i, np.float32)
    w = np.float32(cmap - 0.5)                  # [K,3]
    w1, w2, w3 = _bf_splits(w, 3)
    c2 = np.sum(cmap * cmap, axis=1)            # fp64
    b = (37.5 - 100.0 * c2) / 200.0             # fp64 [K]
    b1, b2, b3 = _bf_splits(np.float32(b), 3)
    # lhsT row i pairs with image row i:
    # 0-2 (w1,p1)  3-5 (w1,p2)  6-8 (w1,p3)  9-11 (w2,p1)  12-14 (w2,p2)
    # 15-17 (w3,p1)  18-20 (b1|b2|b3, ones)
    cmt = np.empty((NROW, K), _BF)
    cmt[0:3] = w1.T
    cmt[3:6] = w1.T
    cmt[6:9] = w1.T
    cmt[9:12] = w2.T
    cmt[12:15] = w2.T
    cmt[15:18] = w3.T
    cmt[18] = b1
    cmt[19] = b2
    cmt[20] = b3
    vmat = np.empty((KH, 4), _BF)
    vmat[:, 0] = 1.0
    vmat[:, 1] = v_i[0:KH].astype(_BF)
    vmat[:, 2] = 1.0
    vmat[:, 3] = v_i[KH:K].astype(_BF)
    return cmt, vmat


def _prep_image_slab(slab):
    """slab: [3, n] float32 -> [2*NROW, n] bf16 rows (A half then B copy)."""
    n = slab.shape[1]
    p1, p2, p3 = _bf_splits(slab, 3)
    img = np.empty((2 * NROW, n), _BF)
    img[0:3] = p1
    img[3:6] = p2
    img[6:9] = p3
    img[9:12] = p1
    img[12:15] = p2
    img[15:18] = p1
    img[18:21] = 1.0
    img[NROW:2 * NROW] = img[0:NROW]
    return img


def _kernel_impl(image, cmap, v_i, _trace=False):
    image = np.ascontiguousarray(np.asarray(image, np.float32))
    cmt, vmat = _prep_consts(cmap, v_i)

    rows_per_core = NB * H // N_CORES          # 256 rows of H per core
    in_maps = []
    for i in range(N_CORES):
        n = (i * rows_per_core) // H
        h0 = (i * rows_per_core) % H
        slab = image[n, :, h0:h0 + rows_per_core, :].reshape(3, -1)
        in_maps.append({"img": _prep_image_slab(slab), "cmt": cmt,
                        "vmat": vmat})

    nc = _get_nc(PIX_PER_CORE)
    res = run_bass_kernel_spmd(nc, in_maps, core_ids=list(range(N_CORES)),
                               trace=_trace)
    out = np.empty((NB, H, W), np.float32)
    for i in range(N_CORES):
        n = (i * rows_per_core) // H
        h0 = (i * rows_per_core) % H
        out[n, h0:h0 + rows_per_core, :] = \
            res.results[i]["vel"].reshape(rows_per_core, W)
    return out, res


def kernel(image, cmap, v_i):
    out, _ = _kernel_impl(image, cmap, v_i)
    return out
